# revision 44
# baseline (speedup 1.0000x reference)
"""Trainium2 Bass kernel for a ResNet Bottleneck block (inference).

Reference computation (NCHW, N=128, Cin=Cout=1024, width=256, H=W=14):
    out = relu(bn1(conv1x1(x, w1)))          # 1024 -> 256
    out = relu(bn2(conv3x3(out, w2, pad=1))) # 256 -> 256
    out = bn3(conv1x1(out, w3))              # 256 -> 1024
    y   = relu(out + x)

Strategy (fp8 DoubleRow):
- Data-parallel: batch 128 sharded as 16 images per NeuronCore (8 cores).
- All convs run as fp8e4 (e4m3) DoubleRow matmuls: 2 fp8 weights/cell double
  the effective contraction to 256/matmul (~1.5x bf16 TFLOP/s at free-dim
  >=196). PSUM accumulates fp32, so precision loss is only operand
  quantization; measured end-to-end rel err ~8e-3 (tol 2e-2).
- BN folded on host into weight scale + bias. Weights are rescaled by
  powers of two (s1=32, s2=2, s3=16) to lift their ~0.02 std out of
  e4m3's subnormal range; ReLU's positive homogeneity carries the scale
  through layers, evictions add correspondingly scaled biases, and the
  host divides the final bf16 output by s1*s2*s3 = 1024.
- conv2 (3x3, pad 1) uses a zero-padded 16x16 per-image SBUF layout; each
  of the 9 taps is one shifted-window DoubleRow matmul per image (moving
  AP [p, ktile=2, row14, col14]; matmul APs allow at most 3 free dims,
  so images can't be paired here).
- Residual + bias3 are folded host-side into xr = 1024*(x + b3), bf16,
  added per PSUM pair as one merged DVE tensor_tensor, ReLU-cast on ACT.
- PSUM is managed as [P, 2, 512] two-bank pair-tiles on two alternating
  tag rings (A/B) over 4-group sub-phases, so each sub-phase's banks get
  a full sub-phase of eviction slack and eviction pairs merge into one op.
- All bulk DMAs ride the sync HWDGE ring in consumption order (FIFO does
  the gating; SWDGE/gpsimd descriptor generation and dep-gated dma_starts
  on the ACT engine both serialize badly). conv2/conv3 are pipelined per
  4-image "super" block; conv1-half1 is emitted after conv2(0) since only
  conv1 needs the full 3.2MB x8 to close any group.
"""

import sys

if "/opt/trn_rl_repo" not in sys.path:
    sys.path.insert(0, "/opt/trn_rl_repo")

import numpy as np
import ml_dtypes

import concourse.bass as bass
import concourse.bacc as bacc
import concourse.tile as tile
from concourse import mybir
from concourse.bass_utils import run_bass_kernel_spmd

EPS = 1e-5
NCORES = 8
NLOC = 16          # images per core
P = 128
C_IN = 1024
WIDTH = 256
C_OUT = 1024
HW = 196           # 14*14
JB = 4             # conv1 contraction double-blocks (1024 = 4*256)
MB3 = 8            # conv3 output 128-blocks
NPAIRS = 8         # image pairs per core
NF = 2 * HW        # 392

S1, S2, S3 = 32.0, 2.0, 16.0
STOT = S1 * S2 * S3            # 1024; fp8 activation scales: out1 32x, out2 64x

BF16 = mybir.dt.bfloat16
F32 = mybir.dt.float32
FP8 = mybir.dt.float8e4
DR = mybir.MatmulPerfMode.DoubleRow
Relu = mybir.ActivationFunctionType.Relu

_cached = {}


def _build():
    """Build + compile the SPMD NEFF (one core's program). Cached."""
    if "nc" in _cached:
        return _cached["nc"]

    nc = bacc.Bacc("TRN2", target_bir_lowering=False, debug=False,
                   num_devices=NCORES)

    # DRAM layouts are exact SBUF images (partition-major), packed host-side.
    # x8 is split by image-half so conv1-half0 can close its accumulation
    # groups after only 1.6MB has landed.
    x8_d = nc.dram_tensor("x8", [2, P, JB * 2 * 8 * HW], FP8,
                          kind="ExternalInput")
    xr_d = nc.dram_tensor("xr", [NPAIRS, P, MB3 * NF], BF16,
                          kind="ExternalInput")
    w1_d = nc.dram_tensor("w1t", [P, JB * 2 * WIDTH], FP8,
                          kind="ExternalInput")
    w2_d = nc.dram_tensor("w2t", [P, 9 * 2 * WIDTH], FP8,
                          kind="ExternalInput")
    w3_d = nc.dram_tensor("w3t", [P, 2 * C_OUT], FP8, kind="ExternalInput")
    b_d = nc.dram_tensor("biases", [P, 4], F32, kind="ExternalInput")
    id_d = nc.dram_tensor("ident", [P, P], BF16, kind="ExternalInput")
    y_d = nc.dram_tensor("y", [NPAIRS, P, MB3 * NF], BF16,
                         kind="ExternalOutput")

    with tile.TileContext(nc) as tc:
        _emit(tc, nc, x8_d, xr_d, w1_d, w2_d, w3_d, b_d, id_d, y_d)

    nc.compile()
    _cached["nc"] = nc
    return nc


def _emit(tc, nc, x8_d, xr_d, w1_d, w2_d, w3_d, b_d, id_d, y_d):
    import contextlib

    Alu = mybir.AluOpType

    with contextlib.ExitStack() as ctx:
        # One SBUF pool (per-tag bufs) + one PSUM pool: every pool adds
        # per-engine drain barriers to the kernel prologue/epilogue.
        sb = ctx.enter_context(tc.tile_pool(name="sb", bufs=1))
        const = xpool = opool = evp = sb
        # PSUM as 4 pair-tiles of [P, 2, 512] f32 (2 banks each = all 8
        # banks): two accumulation groups per tile in separate zero
        # regions, letting eviction pairs merge into one op. Two tags (A/B
        # rings of 2 tiles) alternate across 4-group sub-phases, giving
        # every sub-phase's banks a full sub-phase of eviction slack.
        psp = ctx.enter_context(tc.tile_pool(name="psp", bufs=2, space="PSUM"))

        # ---- Loads -----------------------------------------------------
        # All bulk transfers as 2D [P, contiguous] DMAs on the two HWDGE
        # rings (sync + scalar): SWDGE (gpsimd) descriptor generation is a
        # software loop that delays first-byte by many microseconds.
        # Within a ring, DMAs execute FIFO. x8 gets the sync ring to
        # itself at full bandwidth (it gates conv1); w1+biases lead the
        # scalar ring (needed by the first matmul), while w2/w3/id are
        # dep-gated behind most of x8 so they don't steal bandwidth.
        x8sb = xpool.tile([P, JB, 2, NLOC * HW], FP8, name="x8sb", tag="x8sb")
        HF = 8 * HW
        # SBUF view: [(j kt) block, half, 1568] — each DMA writes one
        # half's j-pair (0.4MB), strided across the two kt block rows.
        x8v = x8sb[:].rearrange("p j k (h f) -> p (j k) h f", h=2)

        w1sb = const.tile([P, JB, 2, WIDTH], FP8, name="w1sb", tag="w1sb")
        nc.scalar.dma_start(w1sb[:].rearrange("p a k c -> p (a k c)"),
                            w1_d.ap())

        ball = const.tile([P, 4], F32, name="ball", tag="ball")
        nc.scalar.dma_start(ball[:], b_d.ap())

        # One DMA per (image-half, contraction j-pair): conv1-half h's
        # j-step consumes exactly one such 0.4MB chunk, in arrival order.
        # Everything rides the sync ring in consumption order — FIFO gates
        # later transfers behind earlier ones with no semaphore deps (a
        # dep-gated dma_start on the scalar engine would block ACT's
        # instruction queue — and with it all its evictions — until the
        # dep fires). PE order is c1h0, c2(0), c1h1, ... so the ring order
        # is x8-h0, w2/w3/id, x8-h1, xr.
        def x8_dmas(half):
            src = x8_d.ap()[half].rearrange("p (b f) -> p b f", b=2 * JB)
            for jj in range(2):        # 0.8MB chunks (two j-pairs each)
                nc.sync.dma_start(x8v[:, 4 * jj:4 * jj + 4, half, :],
                                  src[:, 4 * jj:4 * jj + 4, :])

        x8_dmas(0)

        w2sb = const.tile([P, 9, 2, WIDTH], FP8, name="w2sb", tag="w2sb")
        nc.sync.dma_start(w2sb[:].rearrange("p t k c -> p (t k c)"),
                          w2_d.ap())

        w3sb = const.tile([P, 2, C_OUT], FP8, name="w3sb", tag="w3sb")
        nc.sync.dma_start(w3sb[:].rearrange("p k c -> p (k c)"),
                          w3_d.ap())

        id_t = const.tile([P, P], BF16, name="id_t", tag="id_t")
        nc.sync.dma_start(id_t[:], id_d.ap())

        x8_dmas(1)

        xrsb = xpool.tile([P, NPAIRS, MB3, NF], BF16, name="xrsb", tag="xrsb")
        for np_ in range(NPAIRS):
            dst = xrsb[:, np_, :, :].rearrange("p m f -> p (m f)")
            nc.sync.dma_start(dst, xr_d.ap()[np_])

        # PE warm-up: bridge PE activity from body-start until the first x8
        # block pair lands (HAM keeps warming through conv1's own matmuls).
        scratch = const.tile([P, 512], BF16, name="scratch", tag="scratch")
        nc.gpsimd.memset(scratch[:], 0.0)
        warm_ps = psp.tile([P, 2, 512], F32, name="warm_ps", tag="A")
        for _ in range(12):
            nc.tensor.matmul(warm_ps[:, 0, :], scratch[:, 0:P], scratch[:],
                             start=True, stop=True)

        # Zero-padded conv1 output: per image a 16x16 field per 128-block,
        # payload at rows/cols 1..14. Border zeroing on the otherwise-idle
        # GpSimd so DVE stays free for evictions.
        out1 = opool.tile([P, 2, NLOC, 16, 16], FP8, name="out1", tag="out1")
        o1flat = out1[:].rearrange("p k i r c -> p k (i r c)")
        for half in range(2):
            nc.gpsimd.memset(o1flat[:, :, half * 2048:(half + 1) * 2048], 0.0)

        out2 = opool.tile([P, 2, NLOC * HW], FP8, name="out2", tag="out2")

        tog = [0]

        def evict_relu_bias(dst, src, bias_ap):
            # dst = relu(src + bias), alternating DVE / ACT
            tog[0] ^= 1
            if tog[0]:
                nc.vector.tensor_scalar(dst, src, bias_ap, 0.0, Alu.add,
                                        Alu.max)
            else:
                nc.scalar.activation(dst, src, Relu, bias=bias_ap)

        # ---- conv1 (1x1, 1024->256) + bias + relu -> padded out1 --------
        # Per half: 8 open groups (4 pair-tiles x 2 out-blocks),
        # contraction j outer, groups inner so consecutive matmuls hit
        # different banks.
        def emit_conv1(half):
            nls = range(4)
            pair = {nl: psp.tile([P, 2, 512], F32, name=f"ps1_{nl}",
                                 tag=("A" if nl < 2 else "B")) for nl in nls}
            for j in range(JB):
                for mo in range(2):
                    w_ap = w1sb[:, j, :, mo * P:(mo + 1) * P]
                    for nl in nls:
                        np_ = 4 * half + nl
                        nc.tensor.matmul(
                            pair[nl][:, mo, 0:NF], w_ap,
                            x8sb[:, j, :, np_ * NF:(np_ + 1) * NF],
                            start=(j == 0), stop=(j == JB - 1),
                            perf_mode=DR)
            for nl in nls:
                np_ = 4 * half + nl
                for mo in range(2):
                    dst = out1[:, mo, 2 * np_:2 * np_ + 2, 1:15, 1:15]
                    src = (pair[nl][:, mo, 0:NF]
                           .rearrange("p (i r c) -> p i r c", i=2, r=14))
                    evict_relu_bias(dst, src, ball[:, mo:mo + 1])

        # ---- conv2 + conv3, software-pipelined across super-blocks ------
        # Emission order c2(0), c2(1), c3(0), c2(2), c3(1), ... puts a full
        # conv2 block between conv3(s)'s evictions and the reuse of its
        # PSUM slots, removing the super-boundary bank-recycle stall.
        def emit_conv2(s):
            # conv2 (3x3, 256->256, pad 1), split into two 4-group
            # sub-phases by out-block mo (tags A/B): 2 pair-tiles of 2
            # images each, contraction tap outer so each weight amortizes
            # over 4 matmuls. Per-image matmuls (N=196): the windowed
            # moving AP [p, kt, r, c] is at the 3-free-dim ISA limit.
            # Same-mo pairs share the bias, so evictions merge per pair.
            for mo in range(2):
                p2 = {ip: psp.tile([P, 2, 512], F32, name=f"ps2_{ip}",
                                   tag=("A" if mo == 0 else "B"))
                      for ip in range(2)}
                for tap in range(9):
                    dy, dx = tap // 3, tap % 3
                    w_ap = w2sb[:, tap, :, mo * P:(mo + 1) * P]
                    for ii in range(4):
                        img = 4 * s + ii
                        nc.tensor.matmul(
                            p2[ii // 2][:, ii % 2, 0:HW]
                            .rearrange("p (r c) -> p r c", r=14),
                            w_ap,
                            out1[:, :, img, dy:dy + 14, dx:dx + 14],
                            start=(tap == 0), stop=(tap == 8),
                            perf_mode=DR)
                for ip in range(2):
                    img0 = 4 * s + 2 * ip
                    dst = (out2[:, mo, img0 * HW:(img0 + 2) * HW]
                           .rearrange("p (g f) -> p g f", g=2))
                    evict_relu_bias(dst, p2[ip][:, :, 0:HW],
                                    ball[:, 2 + mo:3 + mo])

        def emit_conv3(s):
            # conv3 (1x1, 256->1024) + residual + relu, four sub-waves of
            # 4 groups (2 m-blocks x 2 pairs). The residual (bias3
            # pre-folded, pre-scaled bf16) is added per pair as one merged
            # DVE tensor_tensor from PSUM, then one merged ACT relu-cast:
            # ~0.86us of engine time per pair buys back 0.35us/pair of PE
            # identity-matmul time, and the PE is the critical engine.
            yst = {nl: evp.tile([P, MB3 * NF], BF16, name=f"yst{nl}",
                                tag="yst", bufs=4) for nl in range(2)}
            on_pe = True   # PE-paced sub-waves beat DVE-paced evictions
            for sw in range(4):            # sub-wave: m-blocks (2sw, 2sw+1)
                p3 = {nl: psp.tile([P, 2, 512], F32, name=f"ps3_{nl}",
                                   tag=("A" if sw % 2 == 0 else "B"))
                      for nl in range(2)}
                for mi in range(2):
                    m = 2 * sw + mi
                    w_ap = w3sb[:, :, m * P:(m + 1) * P]
                    for nl in range(2):
                        np_ = 2 * s + nl
                        nc.tensor.matmul(
                            p3[nl][:, mi, 0:NF], w_ap,
                            out2[:, :, np_ * NF:(np_ + 1) * NF],
                            start=True, stop=not on_pe, perf_mode=DR)
                m0 = 2 * sw
                if on_pe:
                    for nl in range(2):
                        np_ = 2 * s + nl
                        for mi in range(2):
                            nc.tensor.matmul(
                                p3[nl][:, mi, 0:NF], id_t[:],
                                xrsb[:, np_, m0 + mi, :],
                                start=False, stop=True)
                for nl in range(2):
                    np_ = 2 * s + nl
                    dst = (yst[nl][:, m0 * NF:(m0 + 2) * NF]
                           .rearrange("p (g f) -> p g f", g=2))
                    if on_pe:
                        tog[0] ^= 1
                        if tog[0]:
                            nc.vector.tensor_scalar_max(
                                dst, p3[nl][:, :, 0:NF], 0.0)
                        else:
                            nc.scalar.activation(dst, p3[nl][:, :, 0:NF],
                                                 Relu, bias=0.0)
                    else:
                        ts = evp.tile([P, 2, NF], F32, name="tsum",
                                      tag="tsum", bufs=6)
                        nc.vector.tensor_tensor(
                            ts[:], p3[nl][:, :, 0:NF],
                            xrsb[:, np_, m0:m0 + 2, :], Alu.add)
                        nc.scalar.activation(dst, ts[:], Relu, bias=0.0)
                if sw >= 2:
                    # y quarters for the back half overlap remaining compute;
                    # the very last pair is issued from both HWDGE rings in
                    # parallel to shorten the post-eviction tail
                    for nl in range(2):
                        np_ = 2 * s + nl
                        h0, h1 = 2 * sw * NF, (2 * sw + 2) * NF
                        eng = (nc.scalar if (s == 3 and sw == 3 and nl == 0)
                               else nc.sync)
                        eng.dma_start(y_d.ap()[np_][:, h0:h1],
                                      yst[nl][:, h0:h1])
                elif sw == 1:
                    for nl in range(2):
                        np_ = 2 * s + nl
                        nc.sync.dma_start(y_d.ap()[np_][:, 0:4 * NF],
                                          yst[nl][:, 0:4 * NF])

        # conv1-half1 is off the x8 critical path: every conv1 group needs
        # the FULL x8 (contraction over all 1024 channels), but conv2(0)
        # needs only half0's outputs — so it fills the PE while x8's tail
        # would otherwise stall half1, and conv3(s) slots in a full conv2
        # block after its PSUM producers.
        emit_conv1(0)
        emit_conv2(0)
        emit_conv1(1)
        for s in range(1, 4):
            emit_conv2(s)
            emit_conv3(s - 1)
        emit_conv3(3)


def _prep(x, w1, g1, b1, m1, v1, w2, g2, b2, m2, v2, w3, g3, b3, m3, v3):
    """Host-side: fold BN, rescale + quantize to fp8, pack SBUF images."""
    def fold(w, g, b, m, v):
        scale = (g.astype(np.float64) / np.sqrt(v.astype(np.float64) + EPS))
        bias = b.astype(np.float64) - m.astype(np.float64) * scale
        wf = w.astype(np.float64) * scale.reshape(-1, *([1] * (w.ndim - 1)))
        return wf.astype(np.float32), bias.astype(np.float32)

    w1f, bias1 = fold(w1, g1, b1, m1, v1)   # [256,1024,1,1]
    w2f, bias2 = fold(w2, g2, b2, m2, v2)   # [256,256,3,3]
    w3f, bias3 = fold(w3, g3, b3, m3, v3)   # [1024,256,1,1]

    bf = ml_dtypes.bfloat16
    e4 = ml_dtypes.float8_e4m3

    def q8(a):
        return np.clip(a, -240.0, 240.0).astype(e4)

    # lhsT SBUF images [p_in, ..., ktile, co]:
    w1t = q8(np.ascontiguousarray(
        (w1f[:, :, 0, 0] * S1).T.reshape(JB, 2, P, WIDTH)
        .transpose(2, 0, 1, 3).reshape(P, JB * 2 * WIDTH)))
    w2t = q8(np.ascontiguousarray(
        (w2f * S2).transpose(2, 3, 1, 0).reshape(3, 3, 2, P, WIDTH)
        .transpose(3, 0, 1, 2, 4).reshape(P, 9 * 2 * WIDTH)))
    w3t = q8(np.ascontiguousarray(
        (w3f[:, :, 0, 0] * S3).T.reshape(2, P, C_OUT)
        .transpose(1, 0, 2).reshape(P, 2 * C_OUT)))

    b1h = (bias1 * S1).reshape(2, P).T                    # [P, 2]
    b2h = (bias2 * S1 * S2).reshape(2, P).T               # [P, 2] (64x)
    ball = np.ascontiguousarray(
        np.concatenate([b1h, b2h], axis=1), dtype=np.float32)

    # x8: conv1 moving operand, [core][half, P, (j, kt, img8, hw)] fp8
    xs = (x.reshape(NCORES, 2, 8, JB, 2, P, HW)
          .transpose(0, 1, 5, 3, 4, 2, 6)
          .reshape(NCORES, 2, P, JB * 2 * 8 * HW))
    x8 = q8(xs)

    # xr: residual + bias3, pre-scaled: STOT*(x + b3), np-major bf16
    r = x.reshape(NCORES, NLOC, C_OUT, HW) + bias3[None, None, :, None]
    xr = ((r * STOT)
          .reshape(NCORES, NPAIRS, 2, MB3, P, HW)
          .transpose(0, 1, 4, 3, 2, 5)
          .reshape(NCORES, NPAIRS, P, MB3 * NF)).astype(bf)

    common = {"w1t": w1t, "w2t": w2t, "w3t": w3t, "biases": ball,
              "ident": np.eye(P, dtype=np.float32).astype(bf)}
    in_maps = [dict(common, x8=np.ascontiguousarray(x8[i]),
                    xr=np.ascontiguousarray(xr[i]))
               for i in range(NCORES)]
    return in_maps


def kernel(**inputs):
    inputs = {k: np.asarray(v) for k, v in inputs.items()}
    in_maps = _prep(**inputs)
    nc = _build()
    res = run_bass_kernel_spmd(nc, in_maps, core_ids=list(range(NCORES)))

    y = np.empty((NCORES * NLOC, C_OUT, 14, 14), dtype=np.float32)
    for i in range(NCORES):
        r = np.asarray(res.results[i]["y"], dtype=np.float32) / STOT
        r = (r.reshape(NPAIRS, P, MB3, 2, HW)
             .transpose(0, 3, 2, 1, 4)
             .reshape(NLOC, C_OUT, 14, 14))
        y[i * NLOC:(i + 1) * NLOC] = r
    return y
